# revision 1
# baseline (speedup 1.0000x reference)
"""Trainium2 Bass kernel for MeshGenLoss (Chamfer + KL + density-uniformity).

Math:
  d[i,j] = |a_i|^2 + |b_j|^2 - 2 a_i.b_j  is computed as ONE K=33 bf16 matmul
  per [128,512] tile: every fp32 scalar is split into 3 exact bf16 limbs, so
  all 9 limb-products of a.b (plus 3 |a|^2 rows against ones, 3 |b|^2 rows)
  accumulate in fp32 PSUM -> fp32-exact distances at bf16 matmul speed.

  Row-min over 4096 cols: ScalarE evacuates PSUM chunks to bf16 SBUF (with
  a free cast), VectorE runs a bf16 min-tree (2x DVE mode) + final
  reduce-min; job flavors A/B trade ScalarE copies vs direct-PSUM VectorE
  reads to balance the two engines.

Sharding: core c owns rows [512c, 512c+512) of each distance matrix
  (pred->target, target->pred, pred->pred self) for both batches = 24 jobs
  of [128 rows x 4096 cols]. For the self matrix the columns are pre-rotated
  by 512c on the host so the masked diagonal always falls in column-tile 0
  (keeps the SPMD program identical across cores); 1e6*I is added there.
"""

import sys

import ml_dtypes
import numpy as np

sys.path.insert(0, "/opt/trn_rl_repo")

B = 2
N = 4096
L = 512
CORES = 8
ROWS = N // CORES  # 512 rows per core
RB = ROWS // 128  # 4 row blocks per core
CT = N // 512  # 8 column tiles per job
K = 33
BF16 = ml_dtypes.bfloat16
BIG = 3.0e38


def _limbs3(x):
    """Split float64 array into 3 bf16 limbs capturing ~24 significand bits."""
    h = x.astype(BF16)
    r = x - h.astype(np.float64)
    m = r.astype(BF16)
    r2 = r - m.astype(np.float64)
    lo = r2.astype(BF16)
    return h, m, lo


def _build_lhsT(a):
    """a: [n, 3] float64 row points -> lhsT [33, n] bf16.

    Rows 0..26: k=(t,p,q) -> -2 * limb_p(a[:, t])  (repeated over q)
    Rows 27..29: limbs of |a|^2
    Rows 30..32: ones (partner of the |b|^2 rhs rows)
    """
    n = a.shape[0]
    asq = (a * a).sum(-1)
    al = _limbs3(a)  # tuple of [n,3] bf16
    sl = _limbs3(asq)
    out = np.zeros((K, n), dtype=BF16)
    k = 0
    for t in range(3):
        for p in range(3):
            row = (-2.0 * al[p][:, t].astype(np.float64)).astype(BF16)
            for _q in range(3):
                out[k] = row
                k += 1
    for p in range(3):
        out[k] = sl[p]
        k += 1
    for _q in range(3):
        out[k] = np.ones(n, dtype=BF16)
        k += 1
    return out


def _build_rhs(b):
    """b: [m, 3] float64 column points -> rhs [33, m] bf16.

    Rows 0..26: k=(t,p,q) -> limb_q(b[:, t])  (repeated over p)
    Rows 27..29: ones (partner of the |a|^2 lhsT rows)
    Rows 30..32: limbs of |b|^2
    """
    m = b.shape[0]
    bsq = (b * b).sum(-1)
    bl = _limbs3(b)
    sl = _limbs3(bsq)
    out = np.zeros((K, m), dtype=BF16)
    k = 0
    for t in range(3):
        for _p in range(3):
            for q in range(3):
                out[k] = bl[q][:, t]
                k += 1
    for _p in range(3):
        out[k] = np.ones(m, dtype=BF16)
        k += 1
    for q in range(3):
        out[k] = sl[q]
        k += 1
    return out


def _build_program():
    import concourse.bacc as bacc
    import concourse.mybir as mybir
    import concourse.tile as tile
    from contextlib import ExitStack

    dt = mybir.dt
    Alu = mybir.AluOpType
    Act = mybir.ActivationFunctionType

    nc = bacc.Bacc("TRN2", target_bir_lowering=False, debug=False)

    d_lhsT_pt = nc.declare_dram_parameter("lhsT_pt", [B, K, ROWS], dt.bfloat16, isOutput=False)
    d_lhsT_tp = nc.declare_dram_parameter("lhsT_tp", [B, K, ROWS], dt.bfloat16, isOutput=False)
    d_rhs_t = nc.declare_dram_parameter("rhs_t", [B, K, N], dt.bfloat16, isOutput=False)
    d_rhs_p = nc.declare_dram_parameter("rhs_p", [B, K, N], dt.bfloat16, isOutput=False)
    d_diag = nc.declare_dram_parameter("diag", [128, 128], dt.float32, isOutput=False)
    d_mu = nc.declare_dram_parameter("mu_sl", [1, 128], dt.float32, isOutput=False)
    d_lv = nc.declare_dram_parameter("lv_sl", [1, 128], dt.float32, isOutput=False)

    o_pt = nc.declare_dram_parameter("o_pt", [B, RB, 128], dt.float32, isOutput=True)
    o_tp = nc.declare_dram_parameter("o_tp", [B, RB, 128], dt.float32, isOutput=True)
    o_pp = nc.declare_dram_parameter("o_pp", [B, RB, 128], dt.float32, isOutput=True)
    o_kl = nc.declare_dram_parameter("o_kl", [1, 3], dt.float32, isOutput=True)
    o_map = {"pt": o_pt, "tp": o_tp, "pp": o_pp}

    with tile.TileContext(nc) as tc, ExitStack() as ctx:
        consts = ctx.enter_context(tc.tile_pool(name="consts", bufs=1))
        psum = ctx.enter_context(tc.tile_pool(name="psum", bufs=4, space="PSUM"))
        cpool = ctx.enter_context(tc.tile_pool(name="cp", bufs=10))
        apool = ctx.enter_context(tc.tile_pool(name="acc", bufs=24))

        # ---- resident inputs (DMA'd in job-consumption order) --------
        lhsT_sb = {}
        rhs_sb = {}
        def load_rhs(dram, b, tag):
            # leading slice first so the first job's matmuls start ~1.5us
            # earlier; remainder streams behind it
            t = consts.tile([K, N], dt.bfloat16, tag=tag)
            nc.sync.dma_start(out=t[:, :1024], in_=dram[b, :, :1024])
            nc.sync.dma_start(out=t[:, 1024:], in_=dram[b, :, 1024:])
            return t

        for b in range(B):
            t1 = consts.tile([K, ROWS], dt.bfloat16, tag=f"lpt{b}")
            nc.sync.dma_start(out=t1[:], in_=d_lhsT_pt[b])
            lhsT_sb["pt", b] = t1
            lhsT_sb["pp", b] = t1
            rhs_sb["pt", b] = load_rhs(d_rhs_t, b, f"rt{b}")
        for b in range(B):
            t2 = consts.tile([K, ROWS], dt.bfloat16, tag=f"ltp{b}")
            nc.sync.dma_start(out=t2[:], in_=d_lhsT_tp[b])
            lhsT_sb["tp", b] = t2
            r2 = load_rhs(d_rhs_p, b, f"rp{b}")
            rhs_sb["tp", b] = r2
            rhs_sb["pp", b] = r2
        diag_sb = consts.tile([128, 128], dt.float32, tag="diag")
        nc.sync.dma_start(out=diag_sb[:], in_=d_diag[:])
        mu_sb = consts.tile([1, 128], dt.float32, tag="mu")
        nc.sync.dma_start(out=mu_sb[:], in_=d_mu[:])
        lv_sb = consts.tile([1, 128], dt.float32, tag="lv")
        nc.sync.dma_start(out=lv_sb[:], in_=d_lv[:])

        # ---- 24 distance-matrix jobs ---------------------------------
        # Two job flavors balance DVE vs ACT:
        #  A: ScalarE copies all 4 PSUM chunks to bf16 SBUF; VectorE does a
        #     pure-bf16 min tree (2x DVE mode).
        #  B: ScalarE copies only odd chunks; VectorE's level-0 mins read
        #     even chunks straight from PSUM (1x).
        # Jobs grouped by (kind, batch) so early jobs only need the rhs
        # tensor that was DMA'd first.
        jobs = [(b, r, kind) for kind in ("pt", "tp", "pp")
                for b in range(B) for r in range(RB)]
        for jidx, (b, r, kind) in enumerate(jobs):
            lhsT = lhsT_sb[kind, b][:, 128 * r:128 * (r + 1)]
            rhs = rhs_sb[kind, b]
            chunks = []
            for h in range(4):
                ch = psum.tile([128, 1024], dt.float32, tag="ps")
                for t in range(2):
                    nc.tensor.matmul(
                        ch[:, 512 * t:512 * (t + 1)],
                        lhsT, rhs[:, 1024 * h + 512 * t:1024 * h + 512 * (t + 1)],
                        start=True, stop=True,
                    )
                chunks.append(ch)
            if kind == "pp":
                # mask the self-distance diagonal (always in chunk 0 at
                # offset 128*r thanks to the host-side column rotation)
                sl = chunks[0][:, 128 * r:128 * r + 128]
                nc.vector.tensor_tensor(sl, sl, diag_sb[:], Alu.add)
            # first jobs are B-type so VectorE starts after a single copy;
            # none of the DVE-heavy B-jobs in the last stretch
            a_type = jidx not in (0, 5, 7, 9, 11, 13, 17, 19, 21, 23)  # 14 of 24
            if a_type:
                # all 4 chunks into one contiguous bf16 staging buffer ->
                # the whole tree runs as in-place halving on wide 2x TTs
                st = cpool.tile([128, 4096], dt.bfloat16, tag="cp4", bufs=3)
                for h in range(4):
                    nc.scalar.copy(st[:, 1024 * h:1024 * (h + 1)], chunks[h][:])
                nc.vector.tensor_tensor(
                    st[:, :2048], st[:, :2048], st[:, 2048:], Alu.min)
                nc.vector.tensor_tensor(
                    st[:, :1024], st[:, :1024], st[:, 1024:2048], Alu.min)
                m01 = st
            else:
                m01 = cpool.tile([128, 1024], dt.bfloat16, tag="cp")
                m23 = cpool.tile([128, 1024], dt.bfloat16, tag="cp")
                cb1 = cpool.tile([128, 1024], dt.bfloat16, tag="cp")
                nc.scalar.copy(cb1[:], chunks[1][:])
                nc.vector.tensor_tensor(m01[:], chunks[0][:], cb1[:], Alu.min)
                cb3 = cpool.tile([128, 1024], dt.bfloat16, tag="cp")
                nc.scalar.copy(cb3[:], chunks[3][:])
                nc.vector.tensor_tensor(m23[:], chunks[2][:], cb3[:], Alu.min)
                nc.vector.tensor_tensor(m01[:], m01[:], m23[:], Alu.min)
            nc.vector.tensor_tensor(
                m01[:, :512], m01[:, :512], m01[:, 512:1024], Alu.min)
            acc = apool.tile([128, 1], dt.float32, tag="acc")
            nc.vector.tensor_reduce(
                acc[:], m01[:, :512], axis=mybir.AxisListType.X, op=Alu.min)
            nc.sync.dma_start(out=o_map[kind][b, r, :], in_=acc[:, 0])

        # ---- KL partials (at the end: the Exp table-load then overlaps
        # trailing job work instead of delaying the first ACT copies) ----
        s1 = apool.tile([1, 1], dt.float32, tag="kls")
        nc.vector.tensor_reduce(s1[:], lv_sb[:], axis=mybir.AxisListType.X, op=Alu.add)
        e_t = consts.tile([1, 128], dt.float32, tag="klexp")
        s3 = apool.tile([1, 1], dt.float32, tag="kls")
        nc.scalar.activation(e_t[:], lv_sb[:], Act.Exp, accum_out=s3[:])
        sq_t = consts.tile([1, 128], dt.float32, tag="klsq")
        s2 = apool.tile([1, 1], dt.float32, tag="kls")
        nc.scalar.activation(sq_t[:], mu_sb[:], Act.Square, accum_out=s2[:])
        nc.sync.dma_start(out=o_kl[0, 0:1], in_=s1[:, 0])
        nc.sync.dma_start(out=o_kl[0, 1:2], in_=s2[:, 0])
        nc.sync.dma_start(out=o_kl[0, 2:3], in_=s3[:, 0])

    nc.compile()
    return nc


def _make_in_maps(pred, target, mu, logvar):
    pred = np.asarray(pred, dtype=np.float32)
    target = np.asarray(target, dtype=np.float32)
    mu = np.asarray(mu, dtype=np.float32)
    logvar = np.asarray(logvar, dtype=np.float32)

    pred64 = pred.astype(np.float64)
    target64 = target.astype(np.float64)

    # Shared (core-independent) operands
    rhs_t = np.stack([_build_rhs(target64[b]) for b in range(B)])  # [B,K,N]
    rhs_p_full = np.stack([_build_rhs(pred64[b]) for b in range(B)])
    diag = (np.eye(128, dtype=np.float32) * 1.0e6)
    mu_flat = mu.reshape(-1)
    lv_flat = logvar.reshape(-1)

    in_maps = []
    for c in range(CORES):
        rows = slice(ROWS * c, ROWS * (c + 1))
        lhsT_pt = np.stack([_build_lhsT(pred64[b, rows]) for b in range(B)])
        lhsT_tp = np.stack([_build_lhsT(target64[b, rows]) for b in range(B)])
        rot = np.roll(rhs_p_full, -ROWS * c, axis=2)
        in_maps.append({
            "lhsT_pt": lhsT_pt,
            "lhsT_tp": lhsT_tp,
            "rhs_t": rhs_t,
            "rhs_p": np.ascontiguousarray(rot),
            "diag": diag,
            "mu_sl": mu_flat[128 * c:128 * (c + 1)].reshape(1, 128),
            "lv_sl": lv_flat[128 * c:128 * (c + 1)].reshape(1, 128),
        })
    return in_maps


def kernel(pred, target, mu, logvar):
    from concourse.bass_utils import run_bass_kernel_spmd

    in_maps = _make_in_maps(pred, target, mu, logvar)
    nc = _build_program()
    res = run_bass_kernel_spmd(nc, in_maps, list(range(CORES)))
    results = res.results

    nn_pt = np.concatenate([r["o_pt"].reshape(B, ROWS) for r in results], axis=1)
    nn_tp = np.concatenate([r["o_tp"].reshape(B, ROWS) for r in results], axis=1)
    nn_pp = np.concatenate([r["o_pp"].reshape(B, ROWS) for r in results], axis=1)
    kl_parts = np.stack([r["o_kl"].reshape(3) for r in results])  # [CORES,3]

    nn_pt64 = nn_pt.astype(np.float64)
    nn_tp64 = nn_tp.astype(np.float64)
    nn_pp64 = nn_pp.astype(np.float64)

    cd = (nn_pt64.mean(axis=1) + nn_tp64.mean(axis=1)).mean()

    s1 = kl_parts[:, 0].astype(np.float64).sum()
    s2 = kl_parts[:, 1].astype(np.float64).sum()
    s3 = kl_parts[:, 2].astype(np.float64).sum()
    n_kl = B * L
    kl = -0.5 * (n_kl + s1 - s2 - s3) / n_kl

    density = np.std(nn_pp64, axis=1, ddof=1).mean()

    total = cd + 0.001 * kl + 0.1 * density

    return (
        np.float32(total),
        np.float32(cd),
        np.float32(kl),
        np.float32(density),
    )



# revision 3
# speedup vs baseline: 1.0725x; 1.0725x over previous
"""Trainium2 Bass kernel for MeshGenLoss (Chamfer + KL + density-uniformity).

Math: d[i,j] = |a_i|^2 + |b_j|^2 - 2 a_i.b_j as ONE K=33 bf16 matmul per
[128,512] tile (3 exact bf16 limbs per fp32 scalar -> fp32-exact distances
in PSUM at bf16 matmul speed).

v2 structure (vs v1): the pred->target distance matrix is computed ONCE.
Row mins give the pred-side Chamfer term; the target-side term comes from
COLUMN mins: each core accumulates a per-column min slab (bf16, elementwise
TT-min over its 4 row-blocks), DMA-transposes it (xbar), and free-axis
reduces the transposed tile -> per-core partial col-mins, combined on host.
This removes the 8 transposed (target->pred) matrix jobs entirely: PE work
drops from 24 to 16 jobs/core.

Per core: 16 jobs of [128 rows x 4096 cols] (8 pt + 8 pp), each as 2 PSUM
chunks [128,2048] (pool bufs=2 = all 8 banks, 2 chunks lookahead).
 - pt job: ACT copies both chunks -> st bf16 [128,4096]; DVE reduce-min
   (2x mode) -> row min. Batch col-min: 3 chained TT-mins over the 4 st
   tiles -> slab u; dma_start_transpose u -> [128,32,128]; one 3D
   tensor_reduce -> [128,32] partial col-mins.
 - pp job, flavor A (ACT-light): ACT copies chunk0; DVE TT-min(chunk1,
   copy0) -> [128,2048] bf16; DVE reduce-min. Flavor D (DVE-light): ACT
   copies both; DVE reduce-min over [128,4096]. Mix tuned for balance.
 - pp diagonal mask: extra matmul (1000*I)^T @ (1000*I) accumulated into
   the chunk (start=False) adds 1e6 on the (host-rotated) diagonal.

Sharding: core c owns pred/target rows [512c, 512c+512). pp columns are
host-rotated by -512c so the masked diagonal falls in chunk 0 at offset
128r (identical SPMD program on all cores).
"""

import sys

import ml_dtypes
import numpy as np

sys.path.insert(0, "/opt/trn_rl_repo")

B = 2
N = 4096
L = 512
CORES = 8
ROWS = N // CORES  # 512 rows per core
RB = ROWS // 128  # 4 row blocks per core
K = 33
BF16 = ml_dtypes.bfloat16
BIGF = 3.0e38

# per-batch job order: (kind, r) — pt-heavy early so rhs_p DMA can trail
JOB_ORDER = [("pt", 0), ("pt", 1), ("pp", 0), ("pt", 2), ("pp", 1),
             ("pt", 3), ("pp", 2), ("pp", 3)]
# pp flavors: True = A (ACT-light: 1 copy + DVE TT), False = D (2 copies)
PP_FLAVOR_A = {(0, 0): True, (0, 1): True, (0, 2): False, (0, 3): True,
               (1, 0): True, (1, 1): False, (1, 2): True, (1, 3): True}


def _limbs3(x):
    """Split float64 array into 3 bf16 limbs capturing ~24 significand bits."""
    h = x.astype(BF16)
    r = x - h.astype(np.float64)
    m = r.astype(BF16)
    r2 = r - m.astype(np.float64)
    lo = r2.astype(BF16)
    return h, m, lo


def _build_lhsT(a):
    """a: [n, 3] float64 row points -> lhsT [33, n] bf16."""
    n = a.shape[0]
    asq = (a * a).sum(-1)
    al = _limbs3(a)
    sl = _limbs3(asq)
    out = np.zeros((K, n), dtype=BF16)
    k = 0
    for t in range(3):
        for p in range(3):
            row = (-2.0 * al[p][:, t].astype(np.float64)).astype(BF16)
            for _q in range(3):
                out[k] = row
                k += 1
    for p in range(3):
        out[k] = sl[p]
        k += 1
    for _q in range(3):
        out[k] = np.ones(n, dtype=BF16)
        k += 1
    return out


def _build_rhs(b):
    """b: [m, 3] float64 column points -> rhs [33, m] bf16."""
    m = b.shape[0]
    bsq = (b * b).sum(-1)
    bl = _limbs3(b)
    sl = _limbs3(bsq)
    out = np.zeros((K, m), dtype=BF16)
    k = 0
    for t in range(3):
        for _p in range(3):
            for q in range(3):
                out[k] = bl[q][:, t]
                k += 1
    for _p in range(3):
        out[k] = np.ones(m, dtype=BF16)
        k += 1
    for q in range(3):
        out[k] = sl[q]
        k += 1
    return out


def _build_program():
    import concourse.bacc as bacc
    import concourse.mybir as mybir
    import concourse.tile as tile
    from contextlib import ExitStack

    dt = mybir.dt
    Alu = mybir.AluOpType
    Act = mybir.ActivationFunctionType

    nc = bacc.Bacc("TRN2", target_bir_lowering=False, debug=False)

    d_lhsT = nc.declare_dram_parameter("lhsT", [B, K, ROWS], dt.bfloat16, isOutput=False)
    d_rhs_t = nc.declare_dram_parameter("rhs_t", [B, K, N], dt.bfloat16, isOutput=False)
    d_rhs_p = nc.declare_dram_parameter("rhs_p", [B, K, N], dt.bfloat16, isOutput=False)
    d_dql = nc.declare_dram_parameter("dql", [128, 128], dt.bfloat16, isOutput=False)
    d_mu = nc.declare_dram_parameter("mu_sl", [1, 128], dt.float32, isOutput=False)
    d_lv = nc.declare_dram_parameter("lv_sl", [1, 128], dt.float32, isOutput=False)

    # o_min columns: per batch, 8 job rowmins in JOB_ORDER (b0 cols 0-7, b1 8-15)
    o_min = nc.declare_dram_parameter("o_min", [128, 16], dt.float32, isOutput=True)
    o_ct = nc.declare_dram_parameter("o_ct", [B, 128, 32], dt.float32, isOutput=True)
    o_kl = nc.declare_dram_parameter("o_kl", [1, 3], dt.float32, isOutput=True)

    with tile.TileContext(nc) as tc, ExitStack() as ctx:
        consts = ctx.enter_context(tc.tile_pool(name="consts", bufs=1))
        psum = ctx.enter_context(tc.tile_pool(name="psum", bufs=2, space="PSUM"))
        stpool = ctx.enter_context(tc.tile_pool(name="st", bufs=7))
        mpool = ctx.enter_context(tc.tile_pool(name="m", bufs=3))
        upool = ctx.enter_context(tc.tile_pool(name="u", bufs=2))
        utpool = ctx.enter_context(tc.tile_pool(name="ut", bufs=2))

        # ---- resident inputs --------------------------------------------
        # sync queue: lhsT b0, rhs_t b0 (leading 512 first), then b1 tensors
        # gpsimd queue (swdge): rhs_p so pp jobs aren't behind rhs_t
        # scalar queue: tiny consts needed early
        lhsT_sb = {}
        rhs_sb = {}
        for b in range(B):
            t1 = consts.tile([K, ROWS], dt.bfloat16, tag=f"l{b}")
            rt = consts.tile([K, N], dt.bfloat16, tag=f"rt{b}")
            rp = consts.tile([K, N], dt.bfloat16, tag=f"rp{b}")
            lhsT_sb[b] = t1
            rhs_sb["pt", b] = rt
            rhs_sb["pp", b] = rp
        nc.sync.dma_start(out=lhsT_sb[0][:], in_=d_lhsT[0])
        nc.sync.dma_start(out=rhs_sb["pt", 0][:, :512], in_=d_rhs_t[0, :, :512])
        nc.sync.dma_start(out=rhs_sb["pt", 0][:, 512:2048], in_=d_rhs_t[0, :, 512:2048])
        nc.sync.dma_start(out=rhs_sb["pt", 0][:, 2048:], in_=d_rhs_t[0, :, 2048:])
        nc.gpsimd.dma_start(out=rhs_sb["pp", 0][:, :2048], in_=d_rhs_p[0, :, :2048])
        nc.gpsimd.dma_start(out=rhs_sb["pp", 0][:, 2048:], in_=d_rhs_p[0, :, 2048:])
        nc.sync.dma_start(out=lhsT_sb[1][:], in_=d_lhsT[1])
        nc.sync.dma_start(out=rhs_sb["pt", 1][:, :2048], in_=d_rhs_t[1, :, :2048])
        nc.sync.dma_start(out=rhs_sb["pt", 1][:, 2048:], in_=d_rhs_t[1, :, 2048:])
        nc.gpsimd.dma_start(out=rhs_sb["pp", 1][:, :2048], in_=d_rhs_p[1, :, :2048])
        nc.gpsimd.dma_start(out=rhs_sb["pp", 1][:, 2048:], in_=d_rhs_p[1, :, 2048:])
        dql_sb = consts.tile([128, 128], dt.bfloat16, tag="dql")
        nc.scalar.dma_start(out=dql_sb[:], in_=d_dql[:])
        mu_sb = consts.tile([1, 128], dt.float32, tag="mu")
        nc.scalar.dma_start(out=mu_sb[:], in_=d_mu[:])
        lv_sb = consts.tile([1, 128], dt.float32, tag="lv")
        nc.scalar.dma_start(out=lv_sb[:], in_=d_lv[:])

        omin_sb = consts.tile([128, 16], dt.float32, tag="omin")
        ct_sb = {}
        for b in range(B):
            ctt = consts.tile([128, 32], dt.float32, tag=f"ct{b}")
            ct_sb[b] = ctt

        # ---- jobs -------------------------------------------------------
        def make_chunk(lhsT, rhs, h, diag_r=None):
            """One [128,2048] PSUM chunk = 4 matmuls; optional diagonal add."""
            ch = psum.tile([128, 2048], dt.float32, tag="ps")
            for t in range(4):
                c0 = 2048 * h + 512 * t
                nc.tensor.matmul(
                    ch[:, 512 * t:512 * (t + 1)], lhsT, rhs[:, c0:c0 + 512],
                    start=True, stop=not (diag_r is not None and t == 0),
                )
            if diag_r is not None:
                # adds 1e6*I on the host-rotated diagonal (always in h=0, t=0)
                nc.tensor.matmul(
                    ch[:, 128 * diag_r:128 * diag_r + 128], dql_sb[:], dql_sb[:],
                    start=False, stop=True, skip_group_check=True,
                )
            return ch

        st_tiles = {}
        for b in range(B):
            for jidx, (kind, r) in enumerate(JOB_ORDER):
                lhsT = lhsT_sb[b][:, 128 * r:128 * (r + 1)]
                rhs = rhs_sb[kind, b]
                ocol = omin_sb[:, 8 * b + jidx:8 * b + jidx + 1]
                if kind == "pt":
                    c0 = make_chunk(lhsT, rhs, 0)
                    st = stpool.tile([128, 4096], dt.bfloat16, tag="st")
                    nc.scalar.copy(st[:, :2048], c0[:])
                    c1 = make_chunk(lhsT, rhs, 1)
                    nc.scalar.copy(st[:, 2048:], c1[:])
                    nc.vector.tensor_reduce(
                        ocol, st[:], axis=mybir.AxisListType.X, op=Alu.min)
                    st_tiles[b, r] = st
                else:  # pp
                    c0 = make_chunk(lhsT, rhs, 0, diag_r=r)
                    if PP_FLAVOR_A[b, r]:
                        m = mpool.tile([128, 4096], dt.bfloat16, tag="m")
                        nc.scalar.copy(m[:, :2048], c0[:])
                        c1 = make_chunk(lhsT, rhs, 1)
                        nc.vector.tensor_tensor(
                            m[:, 2048:], c1[:], m[:, :2048], Alu.min)
                        nc.vector.tensor_reduce(
                            ocol, m[:, 2048:], axis=mybir.AxisListType.X, op=Alu.min)
                    else:
                        m = mpool.tile([128, 4096], dt.bfloat16, tag="m")
                        nc.scalar.copy(m[:, :2048], c0[:])
                        c1 = make_chunk(lhsT, rhs, 1)
                        nc.scalar.copy(m[:, 2048:], c1[:])
                        nc.vector.tensor_reduce(
                            ocol, m[:], axis=mybir.AxisListType.X, op=Alu.min)

            # ---- batch col-min: merge 4 st tiles, transpose, reduce ----
            u = upool.tile([128, 4096], dt.bfloat16, tag="u")
            nc.vector.tensor_tensor(
                u[:], st_tiles[b, 0][:], st_tiles[b, 1][:], Alu.min)
            nc.vector.tensor_tensor(u[:], u[:], st_tiles[b, 2][:], Alu.min)
            nc.vector.tensor_tensor(u[:], u[:], st_tiles[b, 3][:], Alu.min)
            ut = utpool.tile([128, 32, 128], dt.bfloat16, tag="ut")
            nc.sync.dma_start_transpose(ut[:], u[:])
            ctb = utpool.tile([128, 32], dt.bfloat16, tag="ctb")
            nc.vector.tensor_reduce(
                ctb[:], ut[:], axis=mybir.AxisListType.X, op=Alu.min)
            nc.scalar.copy(ct_sb[b][:], ctb[:])

        # ---- KL partials ------------------------------------------------
        s1 = consts.tile([1, 1], dt.float32, tag="kls1")
        nc.vector.tensor_reduce(s1[:], lv_sb[:], axis=mybir.AxisListType.X, op=Alu.add)
        e_t = consts.tile([1, 128], dt.float32, tag="klexp")
        s3 = consts.tile([1, 1], dt.float32, tag="kls3")
        nc.scalar.activation(e_t[:], lv_sb[:], Act.Exp, accum_out=s3[:])
        sq_t = consts.tile([1, 128], dt.float32, tag="klsq")
        s2 = consts.tile([1, 1], dt.float32, tag="kls2")
        nc.scalar.activation(sq_t[:], mu_sb[:], Act.Square, accum_out=s2[:])

        # ---- outputs ----------------------------------------------------
        nc.sync.dma_start(out=o_min[:], in_=omin_sb[:])
        for b in range(B):
            nc.sync.dma_start(out=o_ct[b], in_=ct_sb[b][:])
        nc.sync.dma_start(out=o_kl[0, 0:1], in_=s1[:, 0])
        nc.sync.dma_start(out=o_kl[0, 1:2], in_=s2[:, 0])
        nc.sync.dma_start(out=o_kl[0, 2:3], in_=s3[:, 0])

    nc.compile()
    return nc


def _make_in_maps(pred, target, mu, logvar):
    pred = np.asarray(pred, dtype=np.float32)
    target = np.asarray(target, dtype=np.float32)
    mu = np.asarray(mu, dtype=np.float32)
    logvar = np.asarray(logvar, dtype=np.float32)

    pred64 = pred.astype(np.float64)
    target64 = target.astype(np.float64)

    rhs_t = np.stack([_build_rhs(target64[b]) for b in range(B)])  # [B,K,N]
    rhs_p_full = np.stack([_build_rhs(pred64[b]) for b in range(B)])
    dql = (np.eye(128) * 1000.0).astype(BF16)
    mu_flat = mu.reshape(-1)
    lv_flat = logvar.reshape(-1)

    in_maps = []
    for c in range(CORES):
        rows = slice(ROWS * c, ROWS * (c + 1))
        lhsT = np.stack([_build_lhsT(pred64[b, rows]) for b in range(B)])
        rot = np.roll(rhs_p_full, -ROWS * c, axis=2)
        in_maps.append({
            "lhsT": lhsT,
            "rhs_t": rhs_t,
            "rhs_p": np.ascontiguousarray(rot),
            "dql": dql,
            "mu_sl": mu_flat[128 * c:128 * (c + 1)].reshape(1, 128),
            "lv_sl": lv_flat[128 * c:128 * (c + 1)].reshape(1, 128),
        })
    return in_maps


def kernel(pred, target, mu, logvar):
    from concourse.bass_utils import run_bass_kernel_spmd

    in_maps = _make_in_maps(pred, target, mu, logvar)
    nc = _build_program()
    res = run_bass_kernel_spmd(nc, in_maps, list(range(CORES)))
    results = res.results

    # unscramble o_min: per batch, columns follow JOB_ORDER
    nn_pt = np.empty((B, N), dtype=np.float64)
    nn_pp = np.empty((B, N), dtype=np.float64)
    for c in range(CORES):
        om = results[c]["o_min"].astype(np.float64)  # [128, 16]
        for b in range(B):
            for jidx, (kind, r) in enumerate(JOB_ORDER):
                rows = slice(ROWS * c + 128 * r, ROWS * c + 128 * r + 128)
                col = om[:, 8 * b + jidx]
                if kind == "pt":
                    nn_pt[b, rows] = col
                else:
                    nn_pp[b, rows] = col

    # col-mins: o_ct[c][b, j_rel, t] = min over core c rows of d(row, 128t+j_rel)
    cts = np.stack([r["o_ct"] for r in results])  # [CORES, B, 128, 32]
    colmin = cts.astype(np.float64).min(axis=0)  # [B, 128, 32]
    nn_tp = colmin.transpose(0, 2, 1).reshape(B, N)  # [b, 128t + j_rel]

    kl_parts = np.stack([r["o_kl"].reshape(3) for r in results])

    cd = (nn_pt.mean(axis=1) + nn_tp.mean(axis=1)).mean()

    s1 = kl_parts[:, 0].astype(np.float64).sum()
    s2 = kl_parts[:, 1].astype(np.float64).sum()
    s3 = kl_parts[:, 2].astype(np.float64).sum()
    n_kl = B * L
    kl = -0.5 * (n_kl + s1 - s2 - s3) / n_kl

    density = np.std(nn_pp, axis=1, ddof=1).mean()

    total = cd + 0.001 * kl + 0.1 * density

    return (
        np.float32(total),
        np.float32(cd),
        np.float32(kl),
        np.float32(density),
    )


# revision 5
# speedup vs baseline: 1.2109x; 1.1290x over previous
"""Trainium2 Bass kernel for MeshGenLoss (Chamfer + KL + density-uniformity).

Math: d[i,j] = |a_i|^2 + |b_j|^2 - 2 a_i.b_j as ONE K=33 bf16 matmul per
[128,512] tile (3 exact bf16 limbs per fp32 scalar -> fp32-exact distances
in PSUM at bf16 matmul speed).

v3 structure: only TWO matrices are computed (pred->target "pt" and the
self matrix "pp"); the transposed direction is never materialized.
 - pt row mins -> pred-side Chamfer term (DVE wide reduce-min per block).
 - pt COLUMN mins -> target-side Chamfer term: per-core column-min slab
   built by chained TT-mins (2x bf16), xbar DMA-transposed, then one 3D
   free-axis reduce -> per-core partial col-mins, min-combined on host.
 - pp is SYMMETRIC, so its row mins == its column mins: pp jobs get NO
   row reduction at all, only the (cheap) column-min machinery. Host
   un-rotates and min-combines across cores.

Per core: 16 jobs of [128 rows x 4096 cols] (8 pt + 8 pp), each as 2 PSUM
chunks [128,2048] (pool bufs=2 = all 8 banks). ACT evacuates chunks to
bf16 st tiles; DVE does the mins. pp diagonal mask: extra matmul
(1000*I)^T @ (1000*I) accumulated into the chunk adds 1e6 on the
(host-rotated) diagonal. Job order staggers pt/pp so each batch's col-min
chain (merge TTs -> DMA transpose -> reduce) overlaps the next stretch of
matmuls instead of trailing at the end.

Sharding: core c owns pred/target rows [512c, 512c+512). pp columns are
host-rotated by -512c so the diagonal falls in chunk 0 at offset 128r
(identical SPMD program on all cores).
"""

import sys

import ml_dtypes
import numpy as np

sys.path.insert(0, "/opt/trn_rl_repo")

B = 2
N = 4096
L = 512
CORES = 8
ROWS = N // CORES  # 512 rows per core
RB = ROWS // 128  # 4 row blocks per core
K = 33
BF16 = ml_dtypes.bfloat16

# per-batch job order: b0 pt-early (rhs_p DMA trails), b1 pp-early (so the
# last colmin chain overlaps the trailing pt stretch)
JOB_ORDER = {
    0: [("pt", 0), ("pt", 1), ("pp", 0), ("pt", 2), ("pp", 1),
        ("pt", 3), ("pp", 2), ("pp", 3)],
    1: [("pp", 0), ("pp", 1), ("pp", 2), ("pp", 3),
        ("pt", 0), ("pt", 1), ("pt", 2), ("pt", 3)],
}


def _limbs3(x):
    """Split float64 array into 3 bf16 limbs capturing ~24 significand bits."""
    h = x.astype(BF16)
    r = x - h.astype(np.float64)
    m = r.astype(BF16)
    r2 = r - m.astype(np.float64)
    lo = r2.astype(BF16)
    return h, m, lo


def _build_lhsT(a):
    """a: [n, 3] float64 row points -> lhsT [33, n] bf16."""
    n = a.shape[0]
    asq = (a * a).sum(-1)
    al = _limbs3(a)
    sl = _limbs3(asq)
    out = np.zeros((K, n), dtype=BF16)
    k = 0
    for t in range(3):
        for p in range(3):
            row = (-2.0 * al[p][:, t].astype(np.float64)).astype(BF16)
            for _q in range(3):
                out[k] = row
                k += 1
    for p in range(3):
        out[k] = sl[p]
        k += 1
    for _q in range(3):
        out[k] = np.ones(n, dtype=BF16)
        k += 1
    return out


def _build_rhs(b):
    """b: [m, 3] float64 column points -> rhs [33, m] bf16."""
    m = b.shape[0]
    bsq = (b * b).sum(-1)
    bl = _limbs3(b)
    sl = _limbs3(bsq)
    out = np.zeros((K, m), dtype=BF16)
    k = 0
    for t in range(3):
        for _p in range(3):
            for q in range(3):
                out[k] = bl[q][:, t]
                k += 1
    for _p in range(3):
        out[k] = np.ones(m, dtype=BF16)
        k += 1
    for q in range(3):
        out[k] = sl[q]
        k += 1
    return out


def _build_program():
    import concourse.bacc as bacc
    import concourse.mybir as mybir
    import concourse.tile as tile
    from contextlib import ExitStack

    dt = mybir.dt
    Alu = mybir.AluOpType
    Act = mybir.ActivationFunctionType

    nc = bacc.Bacc("TRN2", target_bir_lowering=False, debug=False)

    d_lhsT = nc.declare_dram_parameter("lhsT", [B, K, ROWS], dt.bfloat16, isOutput=False)
    d_rhs_t = nc.declare_dram_parameter("rhs_t", [B, K, N], dt.bfloat16, isOutput=False)
    d_rhs_p = nc.declare_dram_parameter("rhs_p", [B, K, N], dt.bfloat16, isOutput=False)
    d_dql = nc.declare_dram_parameter("dql", [128, 128], dt.bfloat16, isOutput=False)
    d_mu = nc.declare_dram_parameter("mu_sl", [1, 128], dt.float32, isOutput=False)
    d_lv = nc.declare_dram_parameter("lv_sl", [1, 128], dt.float32, isOutput=False)

    # o_min: pt rowmins, col = 4*b + r
    o_min = nc.declare_dram_parameter("o_min", [128, 8], dt.float32, isOutput=True)
    # o_ct[kind][b]: transposed colmins; kind 0 = pt (-> nn_tp), 1 = pp
    o_ct = nc.declare_dram_parameter("o_ct", [2, B, 128, 32], dt.float32, isOutput=True)
    o_kl = nc.declare_dram_parameter("o_kl", [1, 3], dt.float32, isOutput=True)

    with tile.TileContext(nc) as tc, ExitStack() as ctx:
        consts = ctx.enter_context(tc.tile_pool(name="consts", bufs=1))
        psum = ctx.enter_context(tc.tile_pool(name="psum", bufs=2, space="PSUM"))
        stpool = ctx.enter_context(tc.tile_pool(name="st", bufs=6))
        slabpool = ctx.enter_context(tc.tile_pool(name="slab", bufs=3))
        utpool = ctx.enter_context(tc.tile_pool(name="ut", bufs=2))

        # ---- resident inputs --------------------------------------------
        lhsT_sb = {}
        rhs_sb = {}
        for b in range(B):
            t1 = consts.tile([K, ROWS], dt.bfloat16, tag=f"l{b}")
            rt = consts.tile([K, N], dt.bfloat16, tag=f"rt{b}")
            rp = consts.tile([K, N], dt.bfloat16, tag=f"rp{b}")
            lhsT_sb[b] = t1
            rhs_sb["pt", b] = rt
            rhs_sb["pp", b] = rp
        # critical path: lhsT r0 slice + first rhs cols, on separate queues
        nc.sync.dma_start(out=lhsT_sb[0][:, :128], in_=d_lhsT[0, :, :128])
        nc.scalar.dma_start(out=rhs_sb["pt", 0][:, :512], in_=d_rhs_t[0, :, :512])
        nc.sync.dma_start(out=lhsT_sb[0][:, 128:], in_=d_lhsT[0, :, 128:])
        nc.scalar.dma_start(out=rhs_sb["pt", 0][:, 512:2048], in_=d_rhs_t[0, :, 512:2048])
        nc.sync.dma_start(out=rhs_sb["pt", 0][:, 2048:], in_=d_rhs_t[0, :, 2048:])
        nc.gpsimd.dma_start(out=rhs_sb["pp", 0][:, :2048], in_=d_rhs_p[0, :, :2048])
        nc.gpsimd.dma_start(out=rhs_sb["pp", 0][:, 2048:], in_=d_rhs_p[0, :, 2048:])
        dql_sb = consts.tile([128, 128], dt.bfloat16, tag="dql")
        nc.scalar.dma_start(out=dql_sb[:], in_=d_dql[:])
        nc.sync.dma_start(out=lhsT_sb[1][:], in_=d_lhsT[1])
        # b1: pp first in job order, so rhs_p[1] before rhs_t[1]
        nc.gpsimd.dma_start(out=rhs_sb["pp", 1][:, :2048], in_=d_rhs_p[1, :, :2048])
        nc.gpsimd.dma_start(out=rhs_sb["pp", 1][:, 2048:], in_=d_rhs_p[1, :, 2048:])
        nc.sync.dma_start(out=rhs_sb["pt", 1][:, :2048], in_=d_rhs_t[1, :, :2048])
        nc.sync.dma_start(out=rhs_sb["pt", 1][:, 2048:], in_=d_rhs_t[1, :, 2048:])
        mu_sb = consts.tile([1, 128], dt.float32, tag="mu")
        nc.scalar.dma_start(out=mu_sb[:], in_=d_mu[:])
        lv_sb = consts.tile([1, 128], dt.float32, tag="lv")
        nc.scalar.dma_start(out=lv_sb[:], in_=d_lv[:])

        omin_sb = consts.tile([128, 8], dt.float32, tag="omin")
        ct_sb = {}
        for kind in ("pt", "pp"):
            for b in range(B):
                ctt = consts.tile([128, 32], dt.float32, tag=f"ct{kind}{b}")
                ct_sb[kind, b] = ctt

        def make_chunk(lhsT, rhs, h, diag_r=None):
            """One [128,2048] PSUM chunk = 4 matmuls; optional diagonal add."""
            ch = psum.tile([128, 2048], dt.float32, tag="ps")
            for t in range(4):
                c0 = 2048 * h + 512 * t
                nc.tensor.matmul(
                    ch[:, 512 * t:512 * (t + 1)], lhsT, rhs[:, c0:c0 + 512],
                    start=True, stop=not (diag_r is not None and t == 0),
                )
            if diag_r is not None:
                nc.tensor.matmul(
                    ch[:, 128 * diag_r:128 * diag_r + 128], dql_sb[:], dql_sb[:],
                    start=False, stop=True, skip_group_check=True,
                )
            return ch

        # per (kind, batch) colmin slab with progressive merge
        slabs = {}
        nmerged = {}

        def merge_into_slab(kind, b, st):
            if (kind, b) not in slabs:
                slabs[kind, b] = st  # first job's st doubles as slab seed
                nmerged[kind, b] = 1
                return
            if nmerged[kind, b] == 1:
                sl = slabpool.tile([128, 4096], dt.bfloat16, tag="sl")
                nc.vector.tensor_tensor(sl[:], slabs[kind, b][:], st[:], Alu.min)
                slabs[kind, b] = sl
            else:
                sl = slabs[kind, b]
                nc.vector.tensor_tensor(sl[:], sl[:], st[:], Alu.min)
            nmerged[kind, b] += 1
            if nmerged[kind, b] == RB:
                sl = slabs[kind, b]
                ut = utpool.tile([128, 32, 128], dt.bfloat16, tag="ut")
                q = nc.sync if kind == "pt" else nc.scalar
                q.dma_start_transpose(ut[:], sl[:])
                ctb = utpool.tile([128, 32], dt.bfloat16, tag="ctb")
                nc.vector.tensor_reduce(
                    ctb[:], ut[:], axis=mybir.AxisListType.X, op=Alu.min)
                nc.scalar.copy(ct_sb[kind, b][:], ctb[:])

        for b in range(B):
            for kind, r in JOB_ORDER[b]:
                lhsT = lhsT_sb[b][:, 128 * r:128 * (r + 1)]
                rhs = rhs_sb[kind, b]
                st = stpool.tile([128, 4096], dt.bfloat16, tag="st")
                c0 = make_chunk(lhsT, rhs, 0, diag_r=r if kind == "pp" else None)
                nc.scalar.copy(st[:, :2048], c0[:])
                c1 = make_chunk(lhsT, rhs, 1)
                nc.scalar.copy(st[:, 2048:], c1[:])
                if kind == "pt":
                    nc.vector.tensor_reduce(
                        omin_sb[:, 4 * b + r:4 * b + r + 1], st[:],
                        axis=mybir.AxisListType.X, op=Alu.min)
                merge_into_slab(kind, b, st)

        # ---- KL partials ------------------------------------------------
        s1 = consts.tile([1, 1], dt.float32, tag="kls1")
        nc.vector.tensor_reduce(s1[:], lv_sb[:], axis=mybir.AxisListType.X, op=Alu.add)
        e_t = consts.tile([1, 128], dt.float32, tag="klexp")
        s3 = consts.tile([1, 1], dt.float32, tag="kls3")
        nc.scalar.activation(e_t[:], lv_sb[:], Act.Exp, accum_out=s3[:])
        sq_t = consts.tile([1, 128], dt.float32, tag="klsq")
        s2 = consts.tile([1, 1], dt.float32, tag="kls2")
        nc.scalar.activation(sq_t[:], mu_sb[:], Act.Square, accum_out=s2[:])

        # ---- outputs ----------------------------------------------------
        nc.sync.dma_start(out=o_min[:], in_=omin_sb[:])
        for ki, kind in enumerate(("pt", "pp")):
            for b in range(B):
                nc.sync.dma_start(out=o_ct[ki, b], in_=ct_sb[kind, b][:])
        nc.sync.dma_start(out=o_kl[0, 0:1], in_=s1[:, 0])
        nc.sync.dma_start(out=o_kl[0, 1:2], in_=s2[:, 0])
        nc.sync.dma_start(out=o_kl[0, 2:3], in_=s3[:, 0])

    nc.compile()
    return nc


def _make_in_maps(pred, target, mu, logvar):
    pred = np.asarray(pred, dtype=np.float32)
    target = np.asarray(target, dtype=np.float32)
    mu = np.asarray(mu, dtype=np.float32)
    logvar = np.asarray(logvar, dtype=np.float32)

    pred64 = pred.astype(np.float64)
    target64 = target.astype(np.float64)

    rhs_t = np.stack([_build_rhs(target64[b]) for b in range(B)])  # [B,K,N]
    rhs_p_full = np.stack([_build_rhs(pred64[b]) for b in range(B)])
    dql = (np.eye(128) * 1000.0).astype(BF16)
    mu_flat = mu.reshape(-1)
    lv_flat = logvar.reshape(-1)

    in_maps = []
    for c in range(CORES):
        rows = slice(ROWS * c, ROWS * (c + 1))
        lhsT = np.stack([_build_lhsT(pred64[b, rows]) for b in range(B)])
        rot = np.roll(rhs_p_full, -ROWS * c, axis=2)
        in_maps.append({
            "lhsT": lhsT,
            "rhs_t": rhs_t,
            "rhs_p": np.ascontiguousarray(rot),
            "dql": dql,
            "mu_sl": mu_flat[128 * c:128 * (c + 1)].reshape(1, 128),
            "lv_sl": lv_flat[128 * c:128 * (c + 1)].reshape(1, 128),
        })
    return in_maps


def kernel(pred, target, mu, logvar):
    from concourse.bass_utils import run_bass_kernel_spmd

    in_maps = _make_in_maps(pred, target, mu, logvar)
    nc = _build_program()
    res = run_bass_kernel_spmd(nc, in_maps, list(range(CORES)))
    results = res.results

    # pt rowmins
    nn_pt = np.empty((B, N), dtype=np.float64)
    for c in range(CORES):
        om = results[c]["o_min"].astype(np.float64)  # [128, 8]
        for b in range(B):
            for r in range(RB):
                rows = slice(ROWS * c + 128 * r, ROWS * c + 128 * r + 128)
                nn_pt[b, rows] = om[:, 4 * b + r]

    # colmins: o_ct[c][ki, b, j_rel, t] = min over core c's rows of col 128t+j_rel
    cts = np.stack([r["o_ct"] for r in results]).astype(np.float64)
    # [CORES, 2, B, 128, 32] -> per-core col vector [CORES, 2, B, 4096]
    colv = cts.transpose(0, 1, 2, 4, 3).reshape(CORES, 2, B, N)
    nn_tp = colv[:, 0].min(axis=0)  # [B, N]
    # pp: un-rotate each core's columns (core c col j' = global (j'+512c)%N)
    pp_parts = np.full((CORES, B, N), np.inf)
    for c in range(CORES):
        pp_parts[c] = np.roll(colv[c, 1], ROWS * c, axis=1)
    nn_pp = pp_parts.min(axis=0)  # [B, N]

    kl_parts = np.stack([r["o_kl"].reshape(3) for r in results])

    cd = (nn_pt.mean(axis=1) + nn_tp.mean(axis=1)).mean()

    s1 = kl_parts[:, 0].astype(np.float64).sum()
    s2 = kl_parts[:, 1].astype(np.float64).sum()
    s3 = kl_parts[:, 2].astype(np.float64).sum()
    n_kl = B * L
    kl = -0.5 * (n_kl + s1 - s2 - s3) / n_kl

    density = np.std(nn_pp, axis=1, ddof=1).mean()

    total = cd + 0.001 * kl + 0.1 * density

    return (
        np.float32(total),
        np.float32(cd),
        np.float32(kl),
        np.float32(density),
    )


# revision 6
# speedup vs baseline: 1.2375x; 1.0219x over previous
"""Trainium2 Bass kernel for MeshGenLoss (Chamfer + KL + density-uniformity).

Math: d[i,j] = |a_i|^2 + |b_j|^2 - 2 a_i.b_j as ONE K=33 bf16 matmul per
[128,512] tile (3 exact bf16 limbs per fp32 scalar -> fp32-exact distances
in PSUM at bf16 matmul speed).

v3 structure: only TWO matrices are computed (pred->target "pt" and the
self matrix "pp"); the transposed direction is never materialized.
 - pt row mins -> pred-side Chamfer term (DVE wide reduce-min per block).
 - pt COLUMN mins -> target-side Chamfer term: per-core column-min slab
   built by chained TT-mins (2x bf16), xbar DMA-transposed, then one 3D
   free-axis reduce -> per-core partial col-mins, min-combined on host.
 - pp is SYMMETRIC, so its row mins == its column mins: pp jobs get NO
   row reduction at all, only the (cheap) column-min machinery. Host
   un-rotates and min-combines across cores.

Per core: 16 jobs of [128 rows x 4096 cols] (8 pt + 8 pp), each as 2 PSUM
chunks [128,2048] (pool bufs=2 = all 8 banks). ACT evacuates chunks to
bf16 st tiles; DVE does the mins. pp diagonal mask: extra matmul
(1000*I)^T @ (1000*I) accumulated into the chunk adds 1e6 on the
(host-rotated) diagonal. Job order staggers pt/pp so each batch's col-min
chain (merge TTs -> DMA transpose -> reduce) overlaps the next stretch of
matmuls instead of trailing at the end.

Sharding: core c owns pred/target rows [512c, 512c+512). pp columns are
host-rotated by -512c so the diagonal falls in chunk 0 at offset 128r
(identical SPMD program on all cores).
"""

import sys

import ml_dtypes
import numpy as np

sys.path.insert(0, "/opt/trn_rl_repo")

B = 2
N = 4096
L = 512
CORES = 8
ROWS = N // CORES  # 512 rows per core
RB = ROWS // 128  # 4 row blocks per core
K = 33
BF16 = ml_dtypes.bfloat16

# per-batch job order: b0 pt-early (rhs_p DMA trails), b1 pp-early (so the
# last colmin chain overlaps the trailing pt stretch)
JOB_ORDER = {
    0: [("pt", 0), ("pt", 1), ("pp", 0), ("pt", 2), ("pp", 1),
        ("pt", 3), ("pp", 2), ("pp", 3)],
    1: [("pp", 0), ("pp", 1), ("pp", 2), ("pp", 3),
        ("pt", 0), ("pt", 1), ("pt", 2), ("pt", 3)],
}


def _limbs3(x):
    """Split float64 array into 3 bf16 limbs capturing ~24 significand bits."""
    h = x.astype(BF16)
    r = x - h.astype(np.float64)
    m = r.astype(BF16)
    r2 = r - m.astype(np.float64)
    lo = r2.astype(BF16)
    return h, m, lo


def _build_lhsT(a):
    """a: [n, 3] float64 row points -> lhsT [33, n] bf16."""
    n = a.shape[0]
    asq = (a * a).sum(-1)
    al = _limbs3(a)
    sl = _limbs3(asq)
    out = np.zeros((K, n), dtype=BF16)
    k = 0
    for t in range(3):
        for p in range(3):
            row = (-2.0 * al[p][:, t].astype(np.float64)).astype(BF16)
            for _q in range(3):
                out[k] = row
                k += 1
    for p in range(3):
        out[k] = sl[p]
        k += 1
    for _q in range(3):
        out[k] = np.ones(n, dtype=BF16)
        k += 1
    return out


def _build_rhs(b):
    """b: [m, 3] float64 column points -> rhs [33, m] bf16."""
    m = b.shape[0]
    bsq = (b * b).sum(-1)
    bl = _limbs3(b)
    sl = _limbs3(bsq)
    out = np.zeros((K, m), dtype=BF16)
    k = 0
    for t in range(3):
        for _p in range(3):
            for q in range(3):
                out[k] = bl[q][:, t]
                k += 1
    for _p in range(3):
        out[k] = np.ones(m, dtype=BF16)
        k += 1
    for q in range(3):
        out[k] = sl[q]
        k += 1
    return out


def _build_program():
    import concourse.bacc as bacc
    import concourse.mybir as mybir
    import concourse.tile as tile
    from contextlib import ExitStack

    dt = mybir.dt
    Alu = mybir.AluOpType
    Act = mybir.ActivationFunctionType

    nc = bacc.Bacc("TRN2", target_bir_lowering=False, debug=False)

    d_lhsT = nc.declare_dram_parameter("lhsT", [B, K, ROWS], dt.bfloat16, isOutput=False)
    d_rhs_t = nc.declare_dram_parameter("rhs_t", [B, K, N], dt.bfloat16, isOutput=False)
    d_rhs_p = nc.declare_dram_parameter("rhs_p", [B, K, N], dt.bfloat16, isOutput=False)
    d_dql = nc.declare_dram_parameter("dql", [128, 128], dt.bfloat16, isOutput=False)
    d_mu = nc.declare_dram_parameter("mu_sl", [1, 128], dt.float32, isOutput=False)
    d_lv = nc.declare_dram_parameter("lv_sl", [1, 128], dt.float32, isOutput=False)

    # o_min: pt rowmins, col = 4*b + r
    o_min = nc.declare_dram_parameter("o_min", [128, 8], dt.float32, isOutput=True)
    # o_ct[kind][b]: transposed colmins; kind 0 = pt (-> nn_tp), 1 = pp
    o_ct = nc.declare_dram_parameter("o_ct", [2, B, 128, 32], dt.float32, isOutput=True)
    o_kl = nc.declare_dram_parameter("o_kl", [1, 3], dt.float32, isOutput=True)

    with tile.TileContext(nc) as tc, ExitStack() as ctx:
        consts = ctx.enter_context(tc.tile_pool(name="consts", bufs=1))
        psum = ctx.enter_context(tc.tile_pool(name="psum", bufs=2, space="PSUM"))
        stpool = ctx.enter_context(tc.tile_pool(name="st", bufs=6))
        slabpool = ctx.enter_context(tc.tile_pool(name="slab", bufs=3))
        utpool = ctx.enter_context(tc.tile_pool(name="ut", bufs=2))

        # ---- resident inputs --------------------------------------------
        lhsT_sb = {}
        rhs_sb = {}
        for b in range(B):
            t1 = consts.tile([K, ROWS], dt.bfloat16, tag=f"l{b}")
            rt = consts.tile([K, N], dt.bfloat16, tag=f"rt{b}")
            rp = consts.tile([K, N], dt.bfloat16, tag=f"rp{b}")
            lhsT_sb[b] = t1
            rhs_sb["pt", b] = rt
            rhs_sb["pp", b] = rp
        # critical path: lhsT r0 slice + first rhs cols, on separate queues
        nc.sync.dma_start(out=lhsT_sb[0][:, :128], in_=d_lhsT[0, :, :128])
        nc.scalar.dma_start(out=rhs_sb["pt", 0][:, :512], in_=d_rhs_t[0, :, :512])
        nc.sync.dma_start(out=lhsT_sb[0][:, 128:], in_=d_lhsT[0, :, 128:])
        nc.scalar.dma_start(out=rhs_sb["pt", 0][:, 512:2048], in_=d_rhs_t[0, :, 512:2048])
        nc.sync.dma_start(out=rhs_sb["pt", 0][:, 2048:], in_=d_rhs_t[0, :, 2048:])
        nc.gpsimd.dma_start(out=rhs_sb["pp", 0][:, :2048], in_=d_rhs_p[0, :, :2048])
        nc.gpsimd.dma_start(out=rhs_sb["pp", 0][:, 2048:], in_=d_rhs_p[0, :, 2048:])
        dql_sb = consts.tile([128, 128], dt.bfloat16, tag="dql")
        nc.scalar.dma_start(out=dql_sb[:], in_=d_dql[:])
        nc.sync.dma_start(out=lhsT_sb[1][:], in_=d_lhsT[1])
        # b1: pp first in job order, so rhs_p[1] before rhs_t[1]
        nc.gpsimd.dma_start(out=rhs_sb["pp", 1][:, :2048], in_=d_rhs_p[1, :, :2048])
        nc.gpsimd.dma_start(out=rhs_sb["pp", 1][:, 2048:], in_=d_rhs_p[1, :, 2048:])
        nc.sync.dma_start(out=rhs_sb["pt", 1][:, :2048], in_=d_rhs_t[1, :, :2048])
        nc.sync.dma_start(out=rhs_sb["pt", 1][:, 2048:], in_=d_rhs_t[1, :, 2048:])
        mu_sb = consts.tile([1, 128], dt.float32, tag="mu")
        nc.scalar.dma_start(out=mu_sb[:], in_=d_mu[:])
        lv_sb = consts.tile([1, 128], dt.float32, tag="lv")
        nc.scalar.dma_start(out=lv_sb[:], in_=d_lv[:])

        omin_sb = consts.tile([128, 8], dt.float32, tag="omin")
        ct_sb = {}
        for kind in ("pt", "pp"):
            for b in range(B):
                ctt = consts.tile([128, 32], dt.float32, tag=f"ct{kind}{b}")
                ct_sb[kind, b] = ctt

        def make_chunk(lhsT, rhs, h, diag_r=None):
            """One [128,2048] PSUM chunk = 4 matmuls; optional diagonal add."""
            ch = psum.tile([128, 2048], dt.float32, tag="ps")
            for t in range(4):
                c0 = 2048 * h + 512 * t
                nc.tensor.matmul(
                    ch[:, 512 * t:512 * (t + 1)], lhsT, rhs[:, c0:c0 + 512],
                    start=True, stop=not (diag_r is not None and t == 0),
                )
            if diag_r is not None:
                nc.tensor.matmul(
                    ch[:, 128 * diag_r:128 * diag_r + 128], dql_sb[:], dql_sb[:],
                    start=False, stop=True, skip_group_check=True,
                )
            return ch

        # per (kind, batch) colmin slab with progressive merge
        slabs = {}
        nmerged = {}

        def merge_into_slab(kind, b, st):
            if (kind, b) not in slabs:
                slabs[kind, b] = st  # first job's st doubles as slab seed
                nmerged[kind, b] = 1
                return
            if nmerged[kind, b] == 1:
                sl = slabpool.tile([128, 4096], dt.bfloat16, tag="sl")
                nc.vector.tensor_tensor(sl[:], slabs[kind, b][:], st[:], Alu.min)
                slabs[kind, b] = sl
            else:
                sl = slabs[kind, b]
                nc.vector.tensor_tensor(sl[:], sl[:], st[:], Alu.min)
            nmerged[kind, b] += 1
            if nmerged[kind, b] == RB:
                sl = slabs[kind, b]
                ut = utpool.tile([128, 32, 128], dt.bfloat16, tag="ut")
                nc.sync.dma_start_transpose(ut[:], sl[:])
                ctb = utpool.tile([128, 32], dt.bfloat16, tag="ctb")
                nc.vector.tensor_reduce(
                    ctb[:], ut[:], axis=mybir.AxisListType.X, op=Alu.min)
                nc.scalar.copy(ct_sb[kind, b][:], ctb[:])

        for b in range(B):
            for kind, r in JOB_ORDER[b]:
                lhsT = lhsT_sb[b][:, 128 * r:128 * (r + 1)]
                rhs = rhs_sb[kind, b]
                st = stpool.tile([128, 4096], dt.bfloat16, tag="st")
                c0 = make_chunk(lhsT, rhs, 0, diag_r=r if kind == "pp" else None)
                nc.scalar.copy(st[:, :2048], c0[:])
                c1 = make_chunk(lhsT, rhs, 1)
                nc.scalar.copy(st[:, 2048:], c1[:])
                if kind == "pt":
                    nc.vector.tensor_reduce(
                        omin_sb[:, 4 * b + r:4 * b + r + 1], st[:],
                        axis=mybir.AxisListType.X, op=Alu.min)
                merge_into_slab(kind, b, st)

        # ---- KL partials ------------------------------------------------
        s1 = consts.tile([1, 1], dt.float32, tag="kls1")
        nc.vector.tensor_reduce(s1[:], lv_sb[:], axis=mybir.AxisListType.X, op=Alu.add)
        e_t = consts.tile([1, 128], dt.float32, tag="klexp")
        s3 = consts.tile([1, 1], dt.float32, tag="kls3")
        nc.scalar.activation(e_t[:], lv_sb[:], Act.Exp, accum_out=s3[:])
        sq_t = consts.tile([1, 128], dt.float32, tag="klsq")
        s2 = consts.tile([1, 1], dt.float32, tag="kls2")
        nc.scalar.activation(sq_t[:], mu_sb[:], Act.Square, accum_out=s2[:])

        # ---- outputs ----------------------------------------------------
        nc.sync.dma_start(out=o_min[:], in_=omin_sb[:])
        for ki, kind in enumerate(("pt", "pp")):
            for b in range(B):
                nc.sync.dma_start(out=o_ct[ki, b], in_=ct_sb[kind, b][:])
        nc.sync.dma_start(out=o_kl[0, 0:1], in_=s1[:, 0])
        nc.sync.dma_start(out=o_kl[0, 1:2], in_=s2[:, 0])
        nc.sync.dma_start(out=o_kl[0, 2:3], in_=s3[:, 0])

    nc.compile()
    return nc


def _make_in_maps(pred, target, mu, logvar):
    pred = np.asarray(pred, dtype=np.float32)
    target = np.asarray(target, dtype=np.float32)
    mu = np.asarray(mu, dtype=np.float32)
    logvar = np.asarray(logvar, dtype=np.float32)

    pred64 = pred.astype(np.float64)
    target64 = target.astype(np.float64)

    rhs_t = np.stack([_build_rhs(target64[b]) for b in range(B)])  # [B,K,N]
    rhs_p_full = np.stack([_build_rhs(pred64[b]) for b in range(B)])
    dql = (np.eye(128) * 1000.0).astype(BF16)
    mu_flat = mu.reshape(-1)
    lv_flat = logvar.reshape(-1)

    in_maps = []
    for c in range(CORES):
        rows = slice(ROWS * c, ROWS * (c + 1))
        lhsT = np.stack([_build_lhsT(pred64[b, rows]) for b in range(B)])
        rot = np.roll(rhs_p_full, -ROWS * c, axis=2)
        in_maps.append({
            "lhsT": lhsT,
            "rhs_t": rhs_t,
            "rhs_p": np.ascontiguousarray(rot),
            "dql": dql,
            "mu_sl": mu_flat[128 * c:128 * (c + 1)].reshape(1, 128),
            "lv_sl": lv_flat[128 * c:128 * (c + 1)].reshape(1, 128),
        })
    return in_maps


def kernel(pred, target, mu, logvar):
    from concourse.bass_utils import run_bass_kernel_spmd

    in_maps = _make_in_maps(pred, target, mu, logvar)
    nc = _build_program()
    res = run_bass_kernel_spmd(nc, in_maps, list(range(CORES)))
    results = res.results

    # pt rowmins
    nn_pt = np.empty((B, N), dtype=np.float64)
    for c in range(CORES):
        om = results[c]["o_min"].astype(np.float64)  # [128, 8]
        for b in range(B):
            for r in range(RB):
                rows = slice(ROWS * c + 128 * r, ROWS * c + 128 * r + 128)
                nn_pt[b, rows] = om[:, 4 * b + r]

    # colmins: o_ct[c][ki, b, j_rel, t] = min over core c's rows of col 128t+j_rel
    cts = np.stack([r["o_ct"] for r in results]).astype(np.float64)
    # [CORES, 2, B, 128, 32] -> per-core col vector [CORES, 2, B, 4096]
    colv = cts.transpose(0, 1, 2, 4, 3).reshape(CORES, 2, B, N)
    nn_tp = colv[:, 0].min(axis=0)  # [B, N]
    # pp: un-rotate each core's columns (core c col j' = global (j'+512c)%N)
    pp_parts = np.full((CORES, B, N), np.inf)
    for c in range(CORES):
        pp_parts[c] = np.roll(colv[c, 1], ROWS * c, axis=1)
    nn_pp = pp_parts.min(axis=0)  # [B, N]

    kl_parts = np.stack([r["o_kl"].reshape(3) for r in results])

    cd = (nn_pt.mean(axis=1) + nn_tp.mean(axis=1)).mean()

    s1 = kl_parts[:, 0].astype(np.float64).sum()
    s2 = kl_parts[:, 1].astype(np.float64).sum()
    s3 = kl_parts[:, 2].astype(np.float64).sum()
    n_kl = B * L
    kl = -0.5 * (n_kl + s1 - s2 - s3) / n_kl

    density = np.std(nn_pp, axis=1, ddof=1).mean()

    total = cd + 0.001 * kl + 0.1 * density

    return (
        np.float32(total),
        np.float32(cd),
        np.float32(kl),
        np.float32(density),
    )


# revision 7
# speedup vs baseline: 1.2885x; 1.0412x over previous
"""Trainium2 Bass kernel for MeshGenLoss (Chamfer + KL + density-uniformity).

Math: d[i,j] = |a_i|^2 + |b_j|^2 - 2 a_i.b_j as ONE K=33 bf16 matmul per
[128,512] tile (3 exact bf16 limbs per fp32 scalar -> fp32-exact distances
in PSUM at bf16 matmul speed).

v3 structure: only TWO matrices are computed (pred->target "pt" and the
self matrix "pp"); the transposed direction is never materialized.
 - pt row mins -> pred-side Chamfer term (DVE wide reduce-min per block).
 - pt COLUMN mins -> target-side Chamfer term: per-core column-min slab
   built by chained TT-mins (2x bf16), xbar DMA-transposed, then one 3D
   free-axis reduce -> per-core partial col-mins, min-combined on host.
 - pp is SYMMETRIC, so its row mins == its column mins: pp jobs get NO
   row reduction at all, only the (cheap) column-min machinery. Host
   un-rotates and min-combines across cores.

Per core: 16 jobs of [128 rows x 4096 cols] (8 pt + 8 pp), each as 2 PSUM
chunks [128,2048] (pool bufs=2 = all 8 banks). ACT evacuates chunks to
bf16 st tiles; DVE does the mins. pp diagonal mask: extra matmul
(1000*I)^T @ (1000*I) accumulated into the chunk adds 1e6 on the
(host-rotated) diagonal. Job order staggers pt/pp so each batch's col-min
chain (merge TTs -> DMA transpose -> reduce) overlaps the next stretch of
matmuls instead of trailing at the end.

Sharding: core c owns pred/target rows [512c, 512c+512). pp columns are
host-rotated by -512c so the diagonal falls in chunk 0 at offset 128r
(identical SPMD program on all cores).
"""

import sys

import ml_dtypes
import numpy as np

sys.path.insert(0, "/opt/trn_rl_repo")

B = 2
N = 4096
L = 512
CORES = 8
ROWS = N // CORES  # 512 rows per core
RB = ROWS // 128  # 4 row blocks per core
K = 33
BF16 = ml_dtypes.bfloat16

# per-batch job order: b0 pt-early (rhs_p DMA trails), b1 pp-early (so the
# last colmin chain overlaps the trailing pt stretch)
JOB_ORDER = {
    0: [("pt", 0), ("pt", 1), ("pp", 0), ("pt", 2), ("pp", 1),
        ("pt", 3), ("pp", 2), ("pp", 3)],
    1: [("pt", 0), ("pp", 0), ("pt", 1), ("pp", 1),
        ("pt", 2), ("pp", 2), ("pt", 3), ("pp", 3)],
}


def _limbs3(x):
    """Split float64 array into 3 bf16 limbs capturing ~24 significand bits."""
    h = x.astype(BF16)
    r = x - h.astype(np.float64)
    m = r.astype(BF16)
    r2 = r - m.astype(np.float64)
    lo = r2.astype(BF16)
    return h, m, lo


def _build_lhsT(a):
    """a: [n, 3] float64 row points -> lhsT [33, n] bf16."""
    n = a.shape[0]
    asq = (a * a).sum(-1)
    al = _limbs3(a)
    sl = _limbs3(asq)
    out = np.zeros((K, n), dtype=BF16)
    k = 0
    for t in range(3):
        for p in range(3):
            row = (-2.0 * al[p][:, t].astype(np.float64)).astype(BF16)
            for _q in range(3):
                out[k] = row
                k += 1
    for p in range(3):
        out[k] = sl[p]
        k += 1
    for _q in range(3):
        out[k] = np.ones(n, dtype=BF16)
        k += 1
    return out


def _build_rhs(b):
    """b: [m, 3] float64 column points -> rhs [33, m] bf16."""
    m = b.shape[0]
    bsq = (b * b).sum(-1)
    bl = _limbs3(b)
    sl = _limbs3(bsq)
    out = np.zeros((K, m), dtype=BF16)
    k = 0
    for t in range(3):
        for _p in range(3):
            for q in range(3):
                out[k] = bl[q][:, t]
                k += 1
    for _p in range(3):
        out[k] = np.ones(m, dtype=BF16)
        k += 1
    for q in range(3):
        out[k] = sl[q]
        k += 1
    return out


def _build_program():
    import concourse.bacc as bacc
    import concourse.mybir as mybir
    import concourse.tile as tile
    from contextlib import ExitStack

    dt = mybir.dt
    Alu = mybir.AluOpType
    Act = mybir.ActivationFunctionType

    nc = bacc.Bacc("TRN2", target_bir_lowering=False, debug=False)

    d_lhsT = nc.declare_dram_parameter("lhsT", [B, K, ROWS], dt.bfloat16, isOutput=False)
    d_rhs_t = nc.declare_dram_parameter("rhs_t", [B, K, N], dt.bfloat16, isOutput=False)
    d_rhs_p = nc.declare_dram_parameter("rhs_p", [B, K, N], dt.bfloat16, isOutput=False)
    d_dql = nc.declare_dram_parameter("dql", [128, 128], dt.bfloat16, isOutput=False)
    d_mu = nc.declare_dram_parameter("mu_sl", [1, 128], dt.float32, isOutput=False)
    d_lv = nc.declare_dram_parameter("lv_sl", [1, 128], dt.float32, isOutput=False)

    # o_min: pt rowmins, col = 4*b + r
    o_min = nc.declare_dram_parameter("o_min", [128, 8], dt.float32, isOutput=True)
    # o_ct[kind][b]: transposed colmins; kind 0 = pt (-> nn_tp), 1 = pp
    o_ct = nc.declare_dram_parameter("o_ct", [2, B, 128, 32], dt.float32, isOutput=True)
    o_kl = nc.declare_dram_parameter("o_kl", [1, 3], dt.float32, isOutput=True)

    with tile.TileContext(nc) as tc, ExitStack() as ctx:
        consts = ctx.enter_context(tc.tile_pool(name="consts", bufs=1))
        psum = ctx.enter_context(tc.tile_pool(name="psum", bufs=2, space="PSUM"))
        stpool = ctx.enter_context(tc.tile_pool(name="st", bufs=6))
        slabpool = ctx.enter_context(tc.tile_pool(name="slab", bufs=3))
        utpool = ctx.enter_context(tc.tile_pool(name="ut", bufs=2))

        # ---- resident inputs --------------------------------------------
        lhsT_sb = {}
        rhs_sb = {}
        for b in range(B):
            t1 = consts.tile([K, ROWS], dt.bfloat16, tag=f"l{b}")
            rt = consts.tile([K, N], dt.bfloat16, tag=f"rt{b}")
            rp = consts.tile([K, N], dt.bfloat16, tag=f"rp{b}")
            lhsT_sb[b] = t1
            rhs_sb["pt", b] = rt
            rhs_sb["pp", b] = rp
        # critical path: lhsT r0 slice + first rhs cols, on separate queues
        nc.sync.dma_start(out=lhsT_sb[0][:, :128], in_=d_lhsT[0, :, :128])
        nc.scalar.dma_start(out=rhs_sb["pt", 0][:, :512], in_=d_rhs_t[0, :, :512])
        nc.sync.dma_start(out=lhsT_sb[0][:, 128:], in_=d_lhsT[0, :, 128:])
        nc.scalar.dma_start(out=rhs_sb["pt", 0][:, 512:2048], in_=d_rhs_t[0, :, 512:2048])
        nc.sync.dma_start(out=rhs_sb["pt", 0][:, 2048:], in_=d_rhs_t[0, :, 2048:])
        nc.gpsimd.dma_start(out=rhs_sb["pp", 0][:, :2048], in_=d_rhs_p[0, :, :2048])
        nc.gpsimd.dma_start(out=rhs_sb["pp", 0][:, 2048:], in_=d_rhs_p[0, :, 2048:])
        dql_sb = consts.tile([128, 128], dt.bfloat16, tag="dql")
        nc.scalar.dma_start(out=dql_sb[:], in_=d_dql[:])
        nc.sync.dma_start(out=lhsT_sb[1][:], in_=d_lhsT[1])
        # b1: pp first in job order, so rhs_p[1] before rhs_t[1]
        nc.gpsimd.dma_start(out=rhs_sb["pp", 1][:, :2048], in_=d_rhs_p[1, :, :2048])
        nc.gpsimd.dma_start(out=rhs_sb["pp", 1][:, 2048:], in_=d_rhs_p[1, :, 2048:])
        nc.sync.dma_start(out=rhs_sb["pt", 1][:, :2048], in_=d_rhs_t[1, :, :2048])
        nc.sync.dma_start(out=rhs_sb["pt", 1][:, 2048:], in_=d_rhs_t[1, :, 2048:])
        mu_sb = consts.tile([1, 128], dt.float32, tag="mu")
        nc.scalar.dma_start(out=mu_sb[:], in_=d_mu[:])
        lv_sb = consts.tile([1, 128], dt.float32, tag="lv")
        nc.scalar.dma_start(out=lv_sb[:], in_=d_lv[:])

        omin_sb = consts.tile([128, 8], dt.float32, tag="omin")
        ct_sb = {}
        for kind in ("pt", "pp"):
            for b in range(B):
                ctt = consts.tile([128, 32], dt.float32, tag=f"ct{kind}{b}")
                ct_sb[kind, b] = ctt

        def make_chunk(lhsT, rhs, h, diag_r=None):
            """One [128,2048] PSUM chunk = 4 matmuls; optional diagonal add."""
            ch = psum.tile([128, 2048], dt.float32, tag="ps")
            for t in range(4):
                c0 = 2048 * h + 512 * t
                nc.tensor.matmul(
                    ch[:, 512 * t:512 * (t + 1)], lhsT, rhs[:, c0:c0 + 512],
                    start=True, stop=not (diag_r is not None and t == 0),
                )
            if diag_r is not None:
                nc.tensor.matmul(
                    ch[:, 128 * diag_r:128 * diag_r + 128], dql_sb[:], dql_sb[:],
                    start=False, stop=True, skip_group_check=True,
                )
            return ch

        # per (kind, batch) colmin slab with progressive merge
        slabs = {}
        nmerged = {}

        def merge_into_slab(kind, b, st):
            if (kind, b) not in slabs:
                slabs[kind, b] = st  # first job's st doubles as slab seed
                nmerged[kind, b] = 1
                return
            if nmerged[kind, b] == 1:
                sl = slabpool.tile([128, 4096], dt.bfloat16, tag="sl")
                nc.vector.tensor_tensor(sl[:], slabs[kind, b][:], st[:], Alu.min)
                slabs[kind, b] = sl
            else:
                sl = slabs[kind, b]
                nc.vector.tensor_tensor(sl[:], sl[:], st[:], Alu.min)
            nmerged[kind, b] += 1
            if nmerged[kind, b] == RB:
                sl = slabs[kind, b]
                ut = utpool.tile([128, 32, 128], dt.bfloat16, tag="ut")
                nc.sync.dma_start_transpose(ut[:], sl[:])
                ctb = utpool.tile([128, 32], dt.bfloat16, tag="ctb")
                nc.vector.tensor_reduce(
                    ctb[:], ut[:], axis=mybir.AxisListType.X, op=Alu.min)
                nc.scalar.copy(ct_sb[kind, b][:], ctb[:])

        for b in range(B):
            for kind, r in JOB_ORDER[b]:
                lhsT = lhsT_sb[b][:, 128 * r:128 * (r + 1)]
                rhs = rhs_sb[kind, b]
                st = stpool.tile([128, 4096], dt.bfloat16, tag="st")
                c0 = make_chunk(lhsT, rhs, 0, diag_r=r if kind == "pp" else None)
                nc.scalar.copy(st[:, :2048], c0[:])
                c1 = make_chunk(lhsT, rhs, 1)
                nc.scalar.copy(st[:, 2048:], c1[:])
                if kind == "pt":
                    nc.vector.tensor_reduce(
                        omin_sb[:, 4 * b + r:4 * b + r + 1], st[:],
                        axis=mybir.AxisListType.X, op=Alu.min)
                merge_into_slab(kind, b, st)

        # ---- KL partials ------------------------------------------------
        s1 = consts.tile([1, 1], dt.float32, tag="kls1")
        nc.vector.tensor_reduce(s1[:], lv_sb[:], axis=mybir.AxisListType.X, op=Alu.add)
        e_t = consts.tile([1, 128], dt.float32, tag="klexp")
        s3 = consts.tile([1, 1], dt.float32, tag="kls3")
        nc.scalar.activation(e_t[:], lv_sb[:], Act.Exp, accum_out=s3[:])
        sq_t = consts.tile([1, 128], dt.float32, tag="klsq")
        s2 = consts.tile([1, 1], dt.float32, tag="kls2")
        nc.scalar.activation(sq_t[:], mu_sb[:], Act.Square, accum_out=s2[:])

        # ---- outputs ----------------------------------------------------
        nc.sync.dma_start(out=o_min[:], in_=omin_sb[:])
        for ki, kind in enumerate(("pt", "pp")):
            for b in range(B):
                nc.sync.dma_start(out=o_ct[ki, b], in_=ct_sb[kind, b][:])
        nc.sync.dma_start(out=o_kl[0, 0:1], in_=s1[:, 0])
        nc.sync.dma_start(out=o_kl[0, 1:2], in_=s2[:, 0])
        nc.sync.dma_start(out=o_kl[0, 2:3], in_=s3[:, 0])

    nc.compile()
    return nc


def _make_in_maps(pred, target, mu, logvar):
    pred = np.asarray(pred, dtype=np.float32)
    target = np.asarray(target, dtype=np.float32)
    mu = np.asarray(mu, dtype=np.float32)
    logvar = np.asarray(logvar, dtype=np.float32)

    pred64 = pred.astype(np.float64)
    target64 = target.astype(np.float64)

    rhs_t = np.stack([_build_rhs(target64[b]) for b in range(B)])  # [B,K,N]
    rhs_p_full = np.stack([_build_rhs(pred64[b]) for b in range(B)])
    dql = (np.eye(128) * 1000.0).astype(BF16)
    mu_flat = mu.reshape(-1)
    lv_flat = logvar.reshape(-1)

    in_maps = []
    for c in range(CORES):
        rows = slice(ROWS * c, ROWS * (c + 1))
        lhsT = np.stack([_build_lhsT(pred64[b, rows]) for b in range(B)])
        rot = np.roll(rhs_p_full, -ROWS * c, axis=2)
        in_maps.append({
            "lhsT": lhsT,
            "rhs_t": rhs_t,
            "rhs_p": np.ascontiguousarray(rot),
            "dql": dql,
            "mu_sl": mu_flat[128 * c:128 * (c + 1)].reshape(1, 128),
            "lv_sl": lv_flat[128 * c:128 * (c + 1)].reshape(1, 128),
        })
    return in_maps


def kernel(pred, target, mu, logvar):
    from concourse.bass_utils import run_bass_kernel_spmd

    in_maps = _make_in_maps(pred, target, mu, logvar)
    nc = _build_program()
    res = run_bass_kernel_spmd(nc, in_maps, list(range(CORES)))
    results = res.results

    # pt rowmins
    nn_pt = np.empty((B, N), dtype=np.float64)
    for c in range(CORES):
        om = results[c]["o_min"].astype(np.float64)  # [128, 8]
        for b in range(B):
            for r in range(RB):
                rows = slice(ROWS * c + 128 * r, ROWS * c + 128 * r + 128)
                nn_pt[b, rows] = om[:, 4 * b + r]

    # colmins: o_ct[c][ki, b, j_rel, t] = min over core c's rows of col 128t+j_rel
    cts = np.stack([r["o_ct"] for r in results]).astype(np.float64)
    # [CORES, 2, B, 128, 32] -> per-core col vector [CORES, 2, B, 4096]
    colv = cts.transpose(0, 1, 2, 4, 3).reshape(CORES, 2, B, N)
    nn_tp = colv[:, 0].min(axis=0)  # [B, N]
    # pp: un-rotate each core's columns (core c col j' = global (j'+512c)%N)
    pp_parts = np.full((CORES, B, N), np.inf)
    for c in range(CORES):
        pp_parts[c] = np.roll(colv[c, 1], ROWS * c, axis=1)
    nn_pp = pp_parts.min(axis=0)  # [B, N]

    kl_parts = np.stack([r["o_kl"].reshape(3) for r in results])

    cd = (nn_pt.mean(axis=1) + nn_tp.mean(axis=1)).mean()

    s1 = kl_parts[:, 0].astype(np.float64).sum()
    s2 = kl_parts[:, 1].astype(np.float64).sum()
    s3 = kl_parts[:, 2].astype(np.float64).sum()
    n_kl = B * L
    kl = -0.5 * (n_kl + s1 - s2 - s3) / n_kl

    density = np.std(nn_pp, axis=1, ddof=1).mean()

    total = cd + 0.001 * kl + 0.1 * density

    return (
        np.float32(total),
        np.float32(cd),
        np.float32(kl),
        np.float32(density),
    )
